# revision 44
# baseline (speedup 1.0000x reference)
"""Trainium2 Bass kernel for nn_MoEALU (soft ripple-carry byte adder), v4.2.

Key restructure vs v3 (validated in sim_v4b.py against the jax reference):
  - Host pre-transposes inputs to [pos, chunk, byte(128), rows] f32 so the
    byte axis lands on SBUF partitions. The nibble segmented sums c[row, 32]
    then run on the (previously idle) PE as fp32 matmuls with the data tile
    as the stationary lhsT [128 bytes, 128 rows] and a fixed 0/1 matrix W
    [128, 32] as the moving rhs, accumulating over 2 byte-chunks into PSUM
    in row-major space. This removes v3's 70us DVE segmented reduce and
    120us Pool low-sum tree.
  - Carries: at temp 100 the carry gates saturate to {0,1} exactly, so
    sigmoids become is_le step compares on Pool (no ACT table reloads);
    gamma' = v0 + (v1-v0)*gamma runs as an f16 tensor_tensor_scan per
    4-tile chain group.
  - All per-tile stages process 2-tile super-tiles to halve per-instruction
    fixed overheads; engines balanced DVE/Pool/ACT; outputs batched 4 tiles
    per DMA (queue-SEQ is held for the whole transfer, so few big DMAs).
Sharding: pure data parallel over batch, 8 cores x 4096 rows.
"""

import numpy as np

B_FULL = 32768
N_CORES = 8
B_CORE = B_FULL // N_CORES  # 4096
P = 128
NT = B_CORE // P  # 32 tiles
NST = NT // 2  # 16 super-tiles (2 tiles each)
GT = 4  # tiles per input DMA row-group
NG = NT // GT  # 8 groups
QT = 4  # tiles per carry-chain group

_BUILT = None


def _make_w():
    w = np.zeros((2, 128, 32), np.float32)
    for c in range(2):
        for byte in range(128):
            full = 128 * c + byte
            h, l = full >> 4, full & 15
            w[c, byte, l] = 1.0  # cols 0..15 -> low sums -> stage 2p
            w[c, byte, 16 + h] = 1.0  # cols 16..31 -> high sums -> stage 2p+1
    return w


def _build():
    import concourse.bass as bass
    import concourse.bacc as bacc
    import concourse.mybir as mybir
    import concourse.tile as tile

    f32 = mybir.dt.float32
    f16 = mybir.dt.float16
    AF = mybir.ActivationFunctionType
    AX = mybir.AxisListType
    OP = mybir.AluOpType

    nc = bacc.Bacc("TRN2", target_bir_lowering=False, debug=False)
    at_d = nc.dram_tensor("at", [4, 2, P, B_CORE], f32, kind="ExternalInput")
    bt_d = nc.dram_tensor("bt", [4, 2, P, B_CORE], f32, kind="ExternalInput")
    w_d = nc.dram_tensor("w", [2, P, 32], f32, kind="ExternalInput")
    out_d = nc.dram_tensor("out", [B_CORE, 4, 256], f16, kind="ExternalOutput")

    def ap(base_ap, off, dims):
        part = base_ap.ap[0]
        return bass.AP(base_ap.tensor, base_ap.offset + off,
                       [list(part)] + [list(d) for d in dims])

    with tile.TileContext(nc) as tc:
        with (
            tc.tile_pool(name="persist", bufs=1) as pp,
            tc.tile_pool(name="pin", bufs=2) as pin,
            tc.tile_pool(name="pa", bufs=3) as pa,
            tc.tile_pool(name="pch", bufs=4) as pch,
            tc.tile_pool(name="pc", bufs=3) as pc,
            tc.tile_pool(name="pout", bufs=3) as pout,
            tc.psum_pool(name="ppsum", bufs=4) as pps,
        ):
            # ---------------- persistent tensors ----------------
            wsb = pp.tile([P, 2, 32], f32, tag="wsb")
            w_base = w_d.ap()
            nc.scalar.dma_start(
                wsb[:].rearrange("p c g -> p (c g)"),
                bass.AP(w_base.tensor, 0, [[32, P], [4096, 2], [1, 32]]))
            un_all = pp.tile([P, NT, 8, 18], f16, tag="un_all")
            msk = pp.tile([P, 256], f16, tag="msk")
            nc.gpsimd.memset(msk[:], 1.0)
            nc.gpsimd.memset(ap(msk[:], 0, [[16, 16]]), 0.0)
            nb100 = pp.tile([P, 1], f32, tag="nb100")
            nc.gpsimd.memset(nb100[:], -100.0)

            # PE warmup: keep the PE busy while the first slabs load so the
            # first real matmuls run at full clock (pstate ramp)
            warm = pps.tile([P, 32], f32, tag="warm")
            wp = warm[:]
            warm_out = bass.AP(wp.tensor, wp.offset,
                               [[list(wp.ap[0])[0], 32], [1, 32]])
            for _ in range(48):
                nc.tensor.matmul(
                    warm_out, ap(wsb[:], 0, [[1, 32]]),
                    ap(wsb[:], 0, [[1, 32]]), start=True, stop=True)

            at_base = at_d.ap()
            bt_base = bt_d.ap()

            def in_view(base, row0, nrows):
                return bass.AP(
                    base.tensor, row0,
                    [[B_CORE, P], [2 * P * B_CORE, 4], [P * B_CORE, 2],
                     [1, nrows]])

            out_base = out_d.ap()

            # --------------- input loading ---------------
            def load_group(g, split_first=False):
                sl = pin.tile([P, 2, 4, 2, GT * P], f32, tag="slab")
                if split_first:
                    # per-tile loads so ST0's matmuls start asap
                    for h in range(4):
                        for tr, base in ((0, at_base), (1, bt_base)):
                            nc.sync.dma_start(
                                ap(sl[:], 4096 * tr + 128 * h,
                                   [[1024, 4], [512, 2], [1, 128]]),
                                in_view(base, GT * P * g + 128 * h, 128))
                else:
                    nc.sync.dma_start(
                        ap(sl[:], 0, [[1, 4096]]),
                        in_view(at_base, GT * P * g, GT * P))
                    nc.sync.dma_start(
                        ap(sl[:], 4096, [[1, 4096]]),
                        in_view(bt_base, GT * P * g, GT * P))
                return sl

            # --------------- phase A per super-tile (2 tiles) ---------------
            # a0: PE matmuls (one step early); a1r: c_sb/m16/ts; a1b: T exp;
            # a2: conv chain (one step late)
            def a0(u, sl):
                jg0 = (2 * u) % GT  # first tile index within load group
                c_ps = pps.tile([P, 2, 2, 8, 16], f32, tag="c")
                for j in range(2):
                    for tr in range(2):
                        for p4 in range(4):
                            for ch in range(2):
                                nc.tensor.matmul(
                                    ap(c_ps[:], 256 * tr + 128 * j + 32 * p4,
                                       [[16, 2], [1, 16]]),
                                    ap(sl[:], 4096 * tr + 1024 * p4 + 512 * ch
                                       + P * (jg0 + j), [[1, P]]),
                                    ap(wsb[:], 32 * ch, [[1, 32]]),
                                    start=(ch == 0), stop=(ch == 1))
                return c_ps

            def a1r_copy(u, c_ps):
                c_sb = pa.tile([P, 512], f32, tag="c_sb")
                nc.scalar.copy(
                    c_sb[:], c_ps[:].rearrange("p a b c e -> p (a b c e)"))
                return c_sb

            def a1r_rest(u, c_sb):
                m16 = pa.tile([P, 32], f32, tag="m16")
                nc.vector.tensor_reduce(
                    m16[:], c_sb[:].rearrange("p (g e) -> p g e", e=16),
                    axis=AX.X, op=OP.max)
                ts = pa.tile([P, 512], f32, tag="ts")
                nc.gpsimd.tensor_sub(
                    ts[:].rearrange("p (g e) -> p g e", e=16),
                    c_sb[:].rearrange("p (g e) -> p g e", e=16),
                    ap(m16[:], 0, [[1, 32], [0, 16]]))
                return ts

            def a1b(u, ts):
                T = pa.tile([P, 2, 2, 8, 16], f16, tag="T")
                nc.scalar.activation(
                    T[:].rearrange("p a b c e -> p (a b c e)"), ts[:],
                    AF.Exp, scale=100.0)
                return T

            def a2(u, T):
                # --- doubled xb + prefix sums (DVE) ---
                xbd = pa.tile([P, 2, 8, 32], f16, tag="xbd")
                nc.vector.tensor_copy(
                    xbd[:].rearrange("p a b (d e) -> p (a b) d e", d=2),
                    ap(T[:], 256, [[16, 16], [0, 2], [1, 16]]))
                p16 = pa.tile([P, 2, 8, 16], f16, tag="p16")
                nc.vector.tensor_tensor_scan(
                    p16[:].rearrange("p a b e -> p (a b e)"),
                    msk[:],
                    ap(T[:], 256, [[1, 256]]),
                    0.0, OP.mult, OP.add)
                # --- conv products q[(j,st) pair, m(17), i(16)] ---
                # pair 0 runs on Pool, pairs 1..15 on DVE; Z1 row on DVE
                q = pa.tile([P, 16, 17, 16], f16, tag="q")
                qt1 = pa.tile([P, 16, 17, 8], f16, tag="qt1")
                qt2 = pa.tile([P, 16, 17, 4], f16, tag="qt2")
                qt3 = pa.tile([P, 16, 17, 2], f16, tag="qt3")
                qr = pa.tile([P, 16, 17], f16, tag="qr")
                nc.vector.tensor_mul(
                    ap(q[:], 256, [[272, 16], [1, 16]]),
                    ap(T[:], 0, [[16, 16], [1, 16]]),
                    ap(p16[:], 15, [[16, 16], [-1, 16]]))
                nc.gpsimd.tensor_mul(
                    ap(q[:], 0, [[16, 16], [1, 16]]),
                    ap(T[:], 0, [[0, 16], [1, 16]]),
                    ap(xbd[:], 16, [[1, 16], [-1, 16]]))
                nc.gpsimd.tensor_add(
                    ap(qt1[:], 0, [[8, 17], [1, 8]]),
                    ap(q[:], 0, [[16, 17], [1, 8]]),
                    ap(q[:], 8, [[16, 17], [1, 8]]))
                nc.gpsimd.tensor_add(
                    ap(qt2[:], 0, [[4, 17], [1, 4]]),
                    ap(qt1[:], 0, [[8, 17], [1, 4]]),
                    ap(qt1[:], 4, [[8, 17], [1, 4]]))
                nc.gpsimd.tensor_add(
                    ap(qt3[:], 0, [[2, 17], [1, 2]]),
                    ap(qt2[:], 0, [[4, 17], [1, 2]]),
                    ap(qt2[:], 2, [[4, 17], [1, 2]]))
                nc.gpsimd.tensor_add(
                    ap(qr[:], 0, [[1, 17]]),
                    ap(qt3[:], 0, [[2, 17]]),
                    ap(qt3[:], 1, [[2, 17]]))
                nc.vector.tensor_mul(
                    ap(q[:], 272, [[272, 15], [16, 16], [1, 16]]),
                    ap(T[:], 16, [[16, 15], [0, 16], [1, 16]]),
                    ap(xbd[:], 48, [[32, 15], [1, 16], [-1, 16]]))
                # --- tree sum over i (pairs 1..15 on DVE; row 16 all) ---
                nc.vector.tensor_add(
                    ap(qt1[:], 136, [[136, 15], [8, 17], [1, 8]]),
                    ap(q[:], 272, [[272, 15], [16, 17], [1, 8]]),
                    ap(q[:], 280, [[272, 15], [16, 17], [1, 8]]))
                nc.vector.tensor_add(
                    ap(qt2[:], 68, [[68, 15], [4, 17], [1, 4]]),
                    ap(qt1[:], 136, [[136, 15], [8, 17], [1, 4]]),
                    ap(qt1[:], 140, [[136, 15], [8, 17], [1, 4]]))
                nc.vector.tensor_add(
                    ap(qt3[:], 34, [[34, 15], [2, 17], [1, 2]]),
                    ap(qt2[:], 68, [[68, 15], [4, 17], [1, 2]]),
                    ap(qt2[:], 70, [[68, 15], [4, 17], [1, 2]]))
                nc.vector.tensor_add(
                    ap(qr[:], 17, [[17, 15], [1, 17]]),
                    ap(qt3[:], 34, [[34, 15], [2, 17]]),
                    ap(qt3[:], 35, [[34, 15], [2, 17]]))
                # --- normalize ---
                Su = pa.tile([P, 16], f32, tag="Su")
                nc.vector.tensor_reduce(
                    Su[:], ap(qr[:], 0, [[136, 2], [17, 8], [1, 16]]),
                    axis=AX.X, op=OP.add)
                rk = pa.tile([P, 16], f32, tag="rk")
                nc.vector.reciprocal(rk[:], Su[:])
                o = 288 * u
                nc.gpsimd.tensor_mul(
                    ap(un_all[:], o + 1, [[144, 2], [18, 8], [1, 17]]),
                    qr[:],
                    ap(rk[:], 0, [[8, 2], [1, 8], [0, 17]]))
                nc.gpsimd.tensor_mul(
                    ap(un_all[:], o, [[144, 2], [18, 8]]),
                    ap(qr[:], 15, [[136, 2], [17, 8]]),
                    ap(rk[:], 0, [[8, 2], [1, 8]]))

            # --------- carry chain for QT tiles [t0, t0+QT) ---------
            def chain(t0):
                o = 144 * t0
                n = 8 * QT
                wap = ap(un_all[:], o + 17, [[144, QT], [18, 8]])
                zap = ap(un_all[:], o + 16, [[144, QT], [18, 8]])
                wz = pch.tile([P, n], f16, tag="wz")
                v0a = pch.tile([P, n], f16, tag="v0a")
                bco = pch.tile([P, n], f16, tag="bco")
                gg = pch.tile([P, n + 1], f16, tag="gg")
                nc.gpsimd.tensor_sub(
                    wz[:].rearrange("p (a b) -> p a b", a=QT), wap, zap)
                nc.gpsimd.tensor_scalar(
                    v0a[:].rearrange("p (a b) -> p a b", a=QT), wap,
                    0.5, None, op0=OP.is_le)
                nc.gpsimd.tensor_scalar(
                    bco[:], wz[:], 0.5, None, op0=OP.is_le)
                nc.gpsimd.tensor_sub(
                    ap(bco[:], 1, [[8, QT], [1, 7]]),
                    ap(bco[:], 1, [[8, QT], [1, 7]]),
                    ap(v0a[:], 1, [[8, QT], [1, 7]]))
                nc.gpsimd.memset(ap(bco[:], 0, [[8, QT]]), 0.0)
                nc.vector.tensor_tensor_scan(
                    ap(gg[:], 1, [[1, n]]),
                    bco[:], v0a[:], 0.0, OP.mult, OP.add)
                nc.gpsimd.memset(ap(gg[:], 0, [[8, QT]]), 0.0)
                g16 = pch.tile([P, QT, 8, 16], f16, tag="g16")
                nc.scalar.copy(
                    g16[:].rearrange("p a b e -> p (a b e)"),
                    ap(gg[:], 0, [[8, QT], [1, 8], [0, 16]]))
                return g16

            # --------- phase C per super-tile, 4 pipelined segments ---------
            obufs = {}

            def c_a(u, g16):
                o = 288 * u
                dlt = pc.tile([P, 2, 8, 16], f16, tag="dlt")
                nc.gpsimd.tensor_sub(
                    dlt[:],
                    ap(un_all[:], o, [[144, 2], [18, 8], [1, 16]]),
                    ap(un_all[:], o + 1, [[144, 2], [18, 8], [1, 16]]))
                tb2 = pc.tile([P, 2, 8, 16], f16, tag="tb2")
                tau = (2 * u) % QT
                nc.gpsimd.tensor_mul(
                    tb2[:], dlt[:],
                    ap(g16[:], 128 * tau, [[128, 2], [16, 8], [1, 16]]))
                sb = pc.tile([P, 2, 8, 16], f16, tag="sb")
                nc.gpsimd.tensor_add(
                    sb[:], tb2[:],
                    ap(un_all[:], o + 1, [[144, 2], [18, 8], [1, 16]]))
                eh = pc.tile([P, 2, 8, 16], f32, tag="eh")
                nc.scalar.activation(
                    eh[:].rearrange("p a b e -> p (a b e)"),
                    sb[:].rearrange("p a b e -> p (a b e)"),
                    AF.Exp, bias=nb100[:], scale=100.0)
                return eh

            def c_b(u, eh):
                ns = pc.tile([P, 16], f32, tag="ns")
                nc.vector.tensor_reduce(
                    ns[:], eh[:].rearrange("p a b e -> p (a b) e"),
                    axis=AX.X, op=OP.add)
                r1 = pc.tile([P, 16], f32, tag="r1")
                nc.vector.reciprocal(r1[:], ns[:])
                A16 = pc.tile([P, 2, 8, 16], f16, tag="A16")
                nc.gpsimd.tensor_mul(
                    A16[:], eh[:], ap(r1[:], 0, [[8, 2], [1, 8], [0, 16]]))
                e2 = pc.tile([P, 2, 8, 16], f32, tag="e2")
                nc.scalar.activation(
                    e2[:].rearrange("p a b e -> p (a b e)"),
                    A16[:].rearrange("p a b e -> p (a b e)"),
                    AF.Exp, bias=nb100[:], scale=100.0)
                return e2

            def c_c(u, e2):
                s2 = pc.tile([P, 16], f32, tag="s2")
                nc.vector.tensor_reduce(
                    s2[:], e2[:].rearrange("p a b e -> p (a b) e"),
                    axis=AX.X, op=OP.add)
                r2 = pc.tile([P, 16], f32, tag="r2")
                nc.vector.reciprocal(r2[:], s2[:])
                e2l = pc.tile([P, 2, 4, 16], f16, tag="e2l")
                nc.gpsimd.tensor_mul(
                    e2l[:],
                    ap(e2[:], 0, [[128, 2], [32, 4], [1, 16]]),
                    ap(r2[:], 0, [[8, 2], [2, 4], [0, 16]]))
                e2h = pc.tile([P, 2, 4, 16], f16, tag="e2h")
                nc.gpsimd.tensor_mul(
                    e2h[:],
                    ap(e2[:], 16, [[128, 2], [32, 4], [1, 16]]),
                    ap(r2[:], 1, [[8, 2], [2, 4], [0, 16]]))
                rep = pc.tile([P, 2, 4, 16, 16], f16, tag="rep")
                nc.scalar.copy(
                    rep[:].rearrange("p a b h l -> p (a b h l)"),
                    ap(e2h[:], 0, [[64, 2], [16, 4], [1, 16], [0, 16]]))
                return rep, e2l

            def c_d(u, rep, e2l):
                if u % 2 == 0:
                    o_t = pout.tile([P, GT, 4, 16, 16], f16, tag="o_t")
                    obufs[u // 2] = o_t
                o_t = obufs[u // 2]
                nc.vector.tensor_mul(
                    ap(o_t[:], 2048 * (u % 2),
                       [[1024, 2], [256, 4], [16, 16], [1, 16]]),
                    rep[:],
                    ap(e2l[:], 0, [[64, 2], [16, 4], [0, 16], [1, 16]]))
                if u % 2 == 1:
                    t0 = 2 * u - 2
                    if u == NST - 1:
                        # split the last store so the tail drains sooner
                        for h in range(2):
                            dv = bass.AP(
                                out_base.tensor, 1024 * P * (t0 + 2 * h),
                                [[1024, P], [1024 * P, 2], [1, 1024]])
                            nc.scalar.dma_start(
                                dv, ap(o_t[:], 2048 * h, [[1, 2048]]))
                    else:
                        dview = bass.AP(
                            out_base.tensor, 1024 * P * t0,
                            [[1024, P], [1024 * P, GT], [1, 1024]])
                        nc.scalar.dma_start(
                            dview,
                            o_t[:].rearrange("p a b h l -> p (a b h l)"))

            # ================= emission schedule =================
            # lags: a1(u) this step; T(u-1) + a2(u-1) next step; chain after
            # a2 of each odd ST; phase_c three steps behind a1.
            g16s = {}
            slabs = {0: load_group(0, split_first=True)}
            c_pss = {0: a0(0, slabs[0])}
            pend = None  # (u, ts) awaiting a1b/a2
            ehs, e2s, reps = {}, {}, {}
            for u in range(NST + 6):
                if pend is not None and pend[0] < NST:
                    pT = a1b(pend[0], pend[1])  # ACT: T(u-1), ready
                if u < NST:
                    csb_u = a1r_copy(u, c_pss.pop(u))  # ACT: c_copy early
                if u - 6 >= 0:
                    c_d(u - 6, *reps.pop(u - 6))
                if u - 5 >= 0 and u - 5 < NST:
                    reps[u - 5] = c_c(u - 5, e2s.pop(u - 5))
                if u - 4 >= 0 and u - 4 < NST:
                    e2s[u - 4] = c_b(u - 4, ehs.pop(u - 4))
                if u < NST:
                    ts_u = a1r_rest(u, csb_u)  # DVE m16; Pool ts
                if u - 3 >= 0 and u - 3 < NST:
                    ehs[u - 3] = c_a(u - 3, g16s[(u - 3) // 2])
                if pend is not None and pend[0] < NST:
                    a2(pend[0], pT)
                    if pend[0] % 2 == 1:
                        t0 = 2 * pend[0] - 2
                        g16s[t0 // QT] = chain(t0)
                    pend = None
                if u < NST:
                    g = (2 * u) // GT
                    if (2 * u) % GT == 0 and g + 1 < NG:
                        slabs[g + 1] = load_group(g + 1)
                    if u + 1 < NST:
                        gn = (2 * (u + 1)) // GT
                        c_pss[u + 1] = a0(u + 1, slabs[gn])
                    pend = (u, ts_u)

    nc.compile()
    return nc


def _get_nc():
    global _BUILT
    if _BUILT is None:
        _BUILT = _build()
    return _BUILT


def kernel(a, b, add_table=None, carry_table=None, b2n=None, n2b=None, **_kw):
    from concourse.bass_utils import run_bass_kernel_spmd

    a = np.asarray(a, dtype=np.float32).reshape(B_FULL, 4, 256)
    b = np.asarray(b, dtype=np.float32).reshape(B_FULL, 4, 256)
    w = _make_w()
    nc = _get_nc()
    in_maps = []
    for i in range(N_CORES):
        sl = slice(i * B_CORE, (i + 1) * B_CORE)
        # [rows, pos, 256] -> [pos, chunk, byte(128), rows]
        at = np.ascontiguousarray(
            a[sl].reshape(B_CORE, 4, 2, P).transpose(1, 2, 3, 0))
        bt = np.ascontiguousarray(
            b[sl].reshape(B_CORE, 4, 2, P).transpose(1, 2, 3, 0))
        in_maps.append({"at": at, "bt": bt, "w": w})
    res = run_bass_kernel_spmd(nc, in_maps, core_ids=list(range(N_CORES)))
    out = np.concatenate([r["out"] for r in res.results], axis=0)
    return out.astype(np.float32)


# revision 45
# speedup vs baseline: 1.0230x; 1.0230x over previous
"""Trainium2 Bass kernel for nn_MoEALU (soft ripple-carry byte adder), v4.2.

Key restructure vs v3 (validated in sim_v4b.py against the jax reference):
  - Host pre-transposes inputs to [pos, chunk, byte(128), rows] f32 so the
    byte axis lands on SBUF partitions. The nibble segmented sums c[row, 32]
    then run on the (previously idle) PE as fp32 matmuls with the data tile
    as the stationary lhsT [128 bytes, 128 rows] and a fixed 0/1 matrix W
    [128, 32] as the moving rhs, accumulating over 2 byte-chunks into PSUM
    in row-major space. This removes v3's 70us DVE segmented reduce and
    120us Pool low-sum tree.
  - Carries: at temp 100 the carry gates saturate to {0,1} exactly, so
    sigmoids become is_le step compares on Pool (no ACT table reloads);
    gamma' = v0 + (v1-v0)*gamma runs as an f16 tensor_tensor_scan per
    4-tile chain group.
  - All per-tile stages process 2-tile super-tiles to halve per-instruction
    fixed overheads; engines balanced DVE/Pool/ACT; outputs batched 4 tiles
    per DMA (queue-SEQ is held for the whole transfer, so few big DMAs).
Sharding: pure data parallel over batch, 8 cores x 4096 rows.
"""

import numpy as np

B_FULL = 32768
N_CORES = 8
B_CORE = B_FULL // N_CORES  # 4096
P = 128
NT = B_CORE // P  # 32 tiles
NST = NT // 2  # 16 super-tiles (2 tiles each)
GT = 4  # tiles per input DMA row-group
NG = NT // GT  # 8 groups
QT = 4  # tiles per carry-chain group

_BUILT = None


def _make_w():
    w = np.zeros((2, 128, 32), np.float32)
    for c in range(2):
        for byte in range(128):
            full = 128 * c + byte
            h, l = full >> 4, full & 15
            w[c, byte, l] = 1.0  # cols 0..15 -> low sums -> stage 2p
            w[c, byte, 16 + h] = 1.0  # cols 16..31 -> high sums -> stage 2p+1
    return w


def _build():
    import concourse.bass as bass
    import concourse.bacc as bacc
    import concourse.mybir as mybir
    import concourse.tile as tile

    f32 = mybir.dt.float32
    f16 = mybir.dt.float16
    AF = mybir.ActivationFunctionType
    AX = mybir.AxisListType
    OP = mybir.AluOpType

    nc = bacc.Bacc("TRN2", target_bir_lowering=False, debug=False)
    at_d = nc.dram_tensor("at", [4, 2, P, B_CORE], f32, kind="ExternalInput")
    bt_d = nc.dram_tensor("bt", [4, 2, P, B_CORE], f32, kind="ExternalInput")
    w_d = nc.dram_tensor("w", [2, P, 32], f32, kind="ExternalInput")
    out_d = nc.dram_tensor("out", [B_CORE, 4, 256], f16, kind="ExternalOutput")

    def ap(base_ap, off, dims):
        part = base_ap.ap[0]
        return bass.AP(base_ap.tensor, base_ap.offset + off,
                       [list(part)] + [list(d) for d in dims])

    with tile.TileContext(nc) as tc:
        with (
            tc.tile_pool(name="persist", bufs=1) as pp,
            tc.tile_pool(name="pin", bufs=2) as pin,
            tc.tile_pool(name="pa", bufs=3) as pa,
            tc.tile_pool(name="pch", bufs=4) as pch,
            tc.tile_pool(name="pc", bufs=3) as pc,
            tc.tile_pool(name="pout", bufs=3) as pout,
            tc.psum_pool(name="ppsum", bufs=4) as pps,
        ):
            # ---------------- persistent tensors ----------------
            wsb = pp.tile([P, 2, 32], f32, tag="wsb")
            w_base = w_d.ap()
            nc.scalar.dma_start(
                wsb[:].rearrange("p c g -> p (c g)"),
                bass.AP(w_base.tensor, 0, [[32, P], [4096, 2], [1, 32]]))
            un_all = pp.tile([P, NT, 8, 18], f16, tag="un_all")
            msk = pp.tile([P, 256], f16, tag="msk")
            nc.gpsimd.memset(msk[:], 1.0)
            nc.gpsimd.memset(ap(msk[:], 0, [[16, 16]]), 0.0)
            nb100 = pp.tile([P, 1], f32, tag="nb100")
            nc.gpsimd.memset(nb100[:], -100.0)

            at_base = at_d.ap()
            bt_base = bt_d.ap()

            def in_view(base, row0, nrows):
                return bass.AP(
                    base.tensor, row0,
                    [[B_CORE, P], [2 * P * B_CORE, 4], [P * B_CORE, 2],
                     [1, nrows]])

            out_base = out_d.ap()

            # --------------- input loading ---------------
            def load_group(g, split_first=False):
                sl = pin.tile([P, 2, 4, 2, GT * P], f32, tag="slab")
                if split_first:
                    # per-tile loads so ST0's matmuls start asap
                    for h in range(4):
                        for tr, base in ((0, at_base), (1, bt_base)):
                            nc.sync.dma_start(
                                ap(sl[:], 4096 * tr + 128 * h,
                                   [[1024, 4], [512, 2], [1, 128]]),
                                in_view(base, GT * P * g + 128 * h, 128))
                else:
                    nc.sync.dma_start(
                        ap(sl[:], 0, [[1, 4096]]),
                        in_view(at_base, GT * P * g, GT * P))
                    nc.sync.dma_start(
                        ap(sl[:], 4096, [[1, 4096]]),
                        in_view(bt_base, GT * P * g, GT * P))
                return sl

            # --------------- phase A per super-tile (2 tiles) ---------------
            # a0: PE matmuls (one step early); a1r: c_sb/m16/ts; a1b: T exp;
            # a2: conv chain (one step late)
            def a0(u, sl):
                jg0 = (2 * u) % GT  # first tile index within load group
                c_ps = pps.tile([P, 2, 2, 8, 16], f32, tag="c")
                for j in range(2):
                    for tr in range(2):
                        for p4 in range(4):
                            for ch in range(2):
                                nc.tensor.matmul(
                                    ap(c_ps[:], 256 * tr + 128 * j + 32 * p4,
                                       [[16, 2], [1, 16]]),
                                    ap(sl[:], 4096 * tr + 1024 * p4 + 512 * ch
                                       + P * (jg0 + j), [[1, P]]),
                                    ap(wsb[:], 32 * ch, [[1, 32]]),
                                    start=(ch == 0), stop=(ch == 1))
                return c_ps

            def a1r_copy(u, c_ps):
                c_sb = pa.tile([P, 512], f32, tag="c_sb")
                nc.scalar.copy(
                    c_sb[:], c_ps[:].rearrange("p a b c e -> p (a b c e)"))
                return c_sb

            def a1r_rest(u, c_sb):
                m16 = pa.tile([P, 32], f32, tag="m16")
                nc.vector.tensor_reduce(
                    m16[:], c_sb[:].rearrange("p (g e) -> p g e", e=16),
                    axis=AX.X, op=OP.max)
                ts = pa.tile([P, 512], f32, tag="ts")
                nc.gpsimd.tensor_sub(
                    ts[:].rearrange("p (g e) -> p g e", e=16),
                    c_sb[:].rearrange("p (g e) -> p g e", e=16),
                    ap(m16[:], 0, [[1, 32], [0, 16]]))
                return ts

            def a1b(u, ts):
                T = pa.tile([P, 2, 2, 8, 16], f16, tag="T")
                nc.scalar.activation(
                    T[:].rearrange("p a b c e -> p (a b c e)"), ts[:],
                    AF.Exp, scale=100.0)
                return T

            def a2(u, T):
                # --- doubled xb + prefix sums (DVE) ---
                xbd = pa.tile([P, 2, 8, 32], f16, tag="xbd")
                nc.vector.tensor_copy(
                    xbd[:].rearrange("p a b (d e) -> p (a b) d e", d=2),
                    ap(T[:], 256, [[16, 16], [0, 2], [1, 16]]))
                p16 = pa.tile([P, 2, 8, 16], f16, tag="p16")
                nc.vector.tensor_tensor_scan(
                    p16[:].rearrange("p a b e -> p (a b e)"),
                    msk[:],
                    ap(T[:], 256, [[1, 256]]),
                    0.0, OP.mult, OP.add)
                # --- conv products q[(j,st) pair, m(17), i(16)] ---
                # pair 0 runs on Pool, pairs 1..15 on DVE; Z1 row on DVE
                q = pa.tile([P, 16, 17, 16], f16, tag="q")
                qt1 = pa.tile([P, 16, 17, 8], f16, tag="qt1")
                qt2 = pa.tile([P, 16, 17, 4], f16, tag="qt2")
                qt3 = pa.tile([P, 16, 17, 2], f16, tag="qt3")
                qr = pa.tile([P, 16, 17], f16, tag="qr")
                nc.vector.tensor_mul(
                    ap(q[:], 256, [[272, 16], [1, 16]]),
                    ap(T[:], 0, [[16, 16], [1, 16]]),
                    ap(p16[:], 15, [[16, 16], [-1, 16]]))
                nc.gpsimd.tensor_mul(
                    ap(q[:], 0, [[16, 16], [1, 16]]),
                    ap(T[:], 0, [[0, 16], [1, 16]]),
                    ap(xbd[:], 16, [[1, 16], [-1, 16]]))
                nc.gpsimd.tensor_add(
                    ap(qt1[:], 0, [[8, 17], [1, 8]]),
                    ap(q[:], 0, [[16, 17], [1, 8]]),
                    ap(q[:], 8, [[16, 17], [1, 8]]))
                nc.gpsimd.tensor_add(
                    ap(qt2[:], 0, [[4, 17], [1, 4]]),
                    ap(qt1[:], 0, [[8, 17], [1, 4]]),
                    ap(qt1[:], 4, [[8, 17], [1, 4]]))
                nc.gpsimd.tensor_add(
                    ap(qt3[:], 0, [[2, 17], [1, 2]]),
                    ap(qt2[:], 0, [[4, 17], [1, 2]]),
                    ap(qt2[:], 2, [[4, 17], [1, 2]]))
                nc.gpsimd.tensor_add(
                    ap(qr[:], 0, [[1, 17]]),
                    ap(qt3[:], 0, [[2, 17]]),
                    ap(qt3[:], 1, [[2, 17]]))
                nc.vector.tensor_mul(
                    ap(q[:], 272, [[272, 15], [16, 16], [1, 16]]),
                    ap(T[:], 16, [[16, 15], [0, 16], [1, 16]]),
                    ap(xbd[:], 48, [[32, 15], [1, 16], [-1, 16]]))
                # --- tree sum over i (pairs 1..15 on DVE; row 16 all) ---
                nc.vector.tensor_add(
                    ap(qt1[:], 136, [[136, 15], [8, 17], [1, 8]]),
                    ap(q[:], 272, [[272, 15], [16, 17], [1, 8]]),
                    ap(q[:], 280, [[272, 15], [16, 17], [1, 8]]))
                nc.vector.tensor_add(
                    ap(qt2[:], 68, [[68, 15], [4, 17], [1, 4]]),
                    ap(qt1[:], 136, [[136, 15], [8, 17], [1, 4]]),
                    ap(qt1[:], 140, [[136, 15], [8, 17], [1, 4]]))
                nc.vector.tensor_add(
                    ap(qt3[:], 34, [[34, 15], [2, 17], [1, 2]]),
                    ap(qt2[:], 68, [[68, 15], [4, 17], [1, 2]]),
                    ap(qt2[:], 70, [[68, 15], [4, 17], [1, 2]]))
                nc.vector.tensor_add(
                    ap(qr[:], 17, [[17, 15], [1, 17]]),
                    ap(qt3[:], 34, [[34, 15], [2, 17]]),
                    ap(qt3[:], 35, [[34, 15], [2, 17]]))
                # --- normalize ---
                Su = pa.tile([P, 16], f32, tag="Su")
                nc.vector.tensor_reduce(
                    Su[:], ap(qr[:], 0, [[136, 2], [17, 8], [1, 16]]),
                    axis=AX.X, op=OP.add)
                rk = pa.tile([P, 16], f32, tag="rk")
                nc.vector.reciprocal(rk[:], Su[:])
                o = 288 * u
                nc.gpsimd.tensor_mul(
                    ap(un_all[:], o + 1, [[144, 2], [18, 8], [1, 17]]),
                    qr[:],
                    ap(rk[:], 0, [[8, 2], [1, 8], [0, 17]]))
                nc.gpsimd.tensor_mul(
                    ap(un_all[:], o, [[144, 2], [18, 8]]),
                    ap(qr[:], 15, [[136, 2], [17, 8]]),
                    ap(rk[:], 0, [[8, 2], [1, 8]]))

            # --------- carry chain for QT tiles [t0, t0+QT) ---------
            def chain(t0):
                o = 144 * t0
                n = 8 * QT
                wap = ap(un_all[:], o + 17, [[144, QT], [18, 8]])
                zap = ap(un_all[:], o + 16, [[144, QT], [18, 8]])
                wz = pch.tile([P, n], f16, tag="wz")
                v0a = pch.tile([P, n], f16, tag="v0a")
                bco = pch.tile([P, n], f16, tag="bco")
                gg = pch.tile([P, n + 1], f16, tag="gg")
                nc.gpsimd.tensor_sub(
                    wz[:].rearrange("p (a b) -> p a b", a=QT), wap, zap)
                nc.gpsimd.tensor_scalar(
                    v0a[:].rearrange("p (a b) -> p a b", a=QT), wap,
                    0.5, None, op0=OP.is_le)
                nc.gpsimd.tensor_scalar(
                    bco[:], wz[:], 0.5, None, op0=OP.is_le)
                nc.gpsimd.tensor_sub(
                    ap(bco[:], 1, [[8, QT], [1, 7]]),
                    ap(bco[:], 1, [[8, QT], [1, 7]]),
                    ap(v0a[:], 1, [[8, QT], [1, 7]]))
                nc.gpsimd.memset(ap(bco[:], 0, [[8, QT]]), 0.0)
                nc.vector.tensor_tensor_scan(
                    ap(gg[:], 1, [[1, n]]),
                    bco[:], v0a[:], 0.0, OP.mult, OP.add)
                nc.gpsimd.memset(ap(gg[:], 0, [[8, QT]]), 0.0)
                g16 = pch.tile([P, QT, 8, 16], f16, tag="g16")
                nc.scalar.copy(
                    g16[:].rearrange("p a b e -> p (a b e)"),
                    ap(gg[:], 0, [[8, QT], [1, 8], [0, 16]]))
                return g16

            # --------- phase C per super-tile, 4 pipelined segments ---------
            obufs = {}

            def c_a(u, g16):
                o = 288 * u
                dlt = pc.tile([P, 2, 8, 16], f16, tag="dlt")
                nc.gpsimd.tensor_sub(
                    dlt[:],
                    ap(un_all[:], o, [[144, 2], [18, 8], [1, 16]]),
                    ap(un_all[:], o + 1, [[144, 2], [18, 8], [1, 16]]))
                tb2 = pc.tile([P, 2, 8, 16], f16, tag="tb2")
                tau = (2 * u) % QT
                nc.gpsimd.tensor_mul(
                    tb2[:], dlt[:],
                    ap(g16[:], 128 * tau, [[128, 2], [16, 8], [1, 16]]))
                sb = pc.tile([P, 2, 8, 16], f16, tag="sb")
                nc.gpsimd.tensor_add(
                    sb[:], tb2[:],
                    ap(un_all[:], o + 1, [[144, 2], [18, 8], [1, 16]]))
                eh = pc.tile([P, 2, 8, 16], f32, tag="eh")
                nc.scalar.activation(
                    eh[:].rearrange("p a b e -> p (a b e)"),
                    sb[:].rearrange("p a b e -> p (a b e)"),
                    AF.Exp, bias=nb100[:], scale=100.0)
                return eh

            def c_b(u, eh):
                ns = pc.tile([P, 16], f32, tag="ns")
                nc.vector.tensor_reduce(
                    ns[:], eh[:].rearrange("p a b e -> p (a b) e"),
                    axis=AX.X, op=OP.add)
                r1 = pc.tile([P, 16], f32, tag="r1")
                nc.vector.reciprocal(r1[:], ns[:])
                A16 = pc.tile([P, 2, 8, 16], f16, tag="A16")
                nc.gpsimd.tensor_mul(
                    A16[:], eh[:], ap(r1[:], 0, [[8, 2], [1, 8], [0, 16]]))
                e2 = pc.tile([P, 2, 8, 16], f32, tag="e2")
                nc.scalar.activation(
                    e2[:].rearrange("p a b e -> p (a b e)"),
                    A16[:].rearrange("p a b e -> p (a b e)"),
                    AF.Exp, bias=nb100[:], scale=100.0)
                return e2

            def c_c(u, e2):
                s2 = pc.tile([P, 16], f32, tag="s2")
                nc.vector.tensor_reduce(
                    s2[:], e2[:].rearrange("p a b e -> p (a b) e"),
                    axis=AX.X, op=OP.add)
                r2 = pc.tile([P, 16], f32, tag="r2")
                nc.vector.reciprocal(r2[:], s2[:])
                e2l = pc.tile([P, 2, 4, 16], f16, tag="e2l")
                nc.gpsimd.tensor_mul(
                    e2l[:],
                    ap(e2[:], 0, [[128, 2], [32, 4], [1, 16]]),
                    ap(r2[:], 0, [[8, 2], [2, 4], [0, 16]]))
                e2h = pc.tile([P, 2, 4, 16], f16, tag="e2h")
                nc.gpsimd.tensor_mul(
                    e2h[:],
                    ap(e2[:], 16, [[128, 2], [32, 4], [1, 16]]),
                    ap(r2[:], 1, [[8, 2], [2, 4], [0, 16]]))
                rep = pc.tile([P, 2, 4, 16, 16], f16, tag="rep")
                nc.scalar.copy(
                    rep[:].rearrange("p a b h l -> p (a b h l)"),
                    ap(e2h[:], 0, [[64, 2], [16, 4], [1, 16], [0, 16]]))
                return rep, e2l

            def c_d(u, rep, e2l):
                if u % 2 == 0:
                    o_t = pout.tile([P, GT, 4, 16, 16], f16, tag="o_t")
                    obufs[u // 2] = o_t
                o_t = obufs[u // 2]
                nc.vector.tensor_mul(
                    ap(o_t[:], 2048 * (u % 2),
                       [[1024, 2], [256, 4], [16, 16], [1, 16]]),
                    rep[:],
                    ap(e2l[:], 0, [[64, 2], [16, 4], [0, 16], [1, 16]]))
                if u % 2 == 1:
                    t0 = 2 * u - 2
                    if u == NST - 1:
                        # split the last store so the tail drains sooner
                        for h in range(2):
                            dv = bass.AP(
                                out_base.tensor, 1024 * P * (t0 + 2 * h),
                                [[1024, P], [1024 * P, 2], [1, 1024]])
                            nc.scalar.dma_start(
                                dv, ap(o_t[:], 2048 * h, [[1, 2048]]))
                    else:
                        dview = bass.AP(
                            out_base.tensor, 1024 * P * t0,
                            [[1024, P], [1024 * P, GT], [1, 1024]])
                        nc.scalar.dma_start(
                            dview,
                            o_t[:].rearrange("p a b h l -> p (a b h l)"))

            # ================= emission schedule =================
            # lags: a1(u) this step; T(u-1) + a2(u-1) next step; chain after
            # a2 of each odd ST; phase_c three steps behind a1.
            g16s = {}
            slabs = {0: load_group(0, split_first=True)}
            c_pss = {0: a0(0, slabs[0])}
            pend = None  # (u, ts) awaiting a1b/a2
            ehs, e2s, reps = {}, {}, {}
            for u in range(NST + 6):
                if pend is not None and pend[0] < NST:
                    pT = a1b(pend[0], pend[1])  # ACT: T(u-1), ready
                if u < NST:
                    csb_u = a1r_copy(u, c_pss.pop(u))  # ACT: c_copy early
                if u - 6 >= 0:
                    c_d(u - 6, *reps.pop(u - 6))
                if u - 5 >= 0 and u - 5 < NST:
                    reps[u - 5] = c_c(u - 5, e2s.pop(u - 5))
                if u - 4 >= 0 and u - 4 < NST:
                    e2s[u - 4] = c_b(u - 4, ehs.pop(u - 4))
                if u < NST:
                    ts_u = a1r_rest(u, csb_u)  # DVE m16; Pool ts
                if u - 3 >= 0 and u - 3 < NST:
                    ehs[u - 3] = c_a(u - 3, g16s[(u - 3) // 2])
                if pend is not None and pend[0] < NST:
                    a2(pend[0], pT)
                    if pend[0] % 2 == 1:
                        t0 = 2 * pend[0] - 2
                        g16s[t0 // QT] = chain(t0)
                    pend = None
                if u < NST:
                    g = (2 * u) // GT
                    if (2 * u) % GT == 0 and g + 1 < NG:
                        slabs[g + 1] = load_group(g + 1)
                    if u + 1 < NST:
                        gn = (2 * (u + 1)) // GT
                        c_pss[u + 1] = a0(u + 1, slabs[gn])
                    pend = (u, ts_u)

    nc.compile()
    return nc


def _get_nc():
    global _BUILT
    if _BUILT is None:
        _BUILT = _build()
    return _BUILT


def kernel(a, b, add_table=None, carry_table=None, b2n=None, n2b=None, **_kw):
    from concourse.bass_utils import run_bass_kernel_spmd

    a = np.asarray(a, dtype=np.float32).reshape(B_FULL, 4, 256)
    b = np.asarray(b, dtype=np.float32).reshape(B_FULL, 4, 256)
    w = _make_w()
    nc = _get_nc()
    in_maps = []
    for i in range(N_CORES):
        sl = slice(i * B_CORE, (i + 1) * B_CORE)
        # [rows, pos, 256] -> [pos, chunk, byte(128), rows]
        at = np.ascontiguousarray(
            a[sl].reshape(B_CORE, 4, 2, P).transpose(1, 2, 3, 0))
        bt = np.ascontiguousarray(
            b[sl].reshape(B_CORE, 4, 2, P).transpose(1, 2, 3, 0))
        in_maps.append({"at": at, "bt": bt, "w": w})
    res = run_bass_kernel_spmd(nc, in_maps, core_ids=list(range(N_CORES)))
    out = np.concatenate([r["out"] for r in res.results], axis=0)
    return out.astype(np.float32)


# revision 46
# speedup vs baseline: 1.0584x; 1.0346x over previous
"""Trainium2 Bass kernel for nn_MoEALU (soft ripple-carry byte adder), v4.2.

Key restructure vs v3 (validated in sim_v4b.py against the jax reference):
  - Host pre-transposes inputs to [pos, chunk, byte(128), rows] f32 so the
    byte axis lands on SBUF partitions. The nibble segmented sums c[row, 32]
    then run on the (previously idle) PE as fp32 matmuls with the data tile
    as the stationary lhsT [128 bytes, 128 rows] and a fixed 0/1 matrix W
    [128, 32] as the moving rhs, accumulating over 2 byte-chunks into PSUM
    in row-major space. This removes v3's 70us DVE segmented reduce and
    120us Pool low-sum tree.
  - Carries: at temp 100 the carry gates saturate to {0,1} exactly, so
    sigmoids become is_le step compares on Pool (no ACT table reloads);
    gamma' = v0 + (v1-v0)*gamma runs as an f16 tensor_tensor_scan per
    4-tile chain group.
  - All per-tile stages process 2-tile super-tiles to halve per-instruction
    fixed overheads; engines balanced DVE/Pool/ACT; outputs batched 4 tiles
    per DMA (queue-SEQ is held for the whole transfer, so few big DMAs).
Sharding: pure data parallel over batch, 8 cores x 4096 rows.
"""

import numpy as np

B_FULL = 32768
N_CORES = 8
B_CORE = B_FULL // N_CORES  # 4096
P = 128
NT = B_CORE // P  # 32 tiles
NST = NT // 2  # 16 super-tiles (2 tiles each)
GT = 4  # tiles per input DMA row-group
NG = NT // GT  # 8 groups
QT = 4  # tiles per carry-chain group

_BUILT = None


def _make_w():
    w = np.zeros((2, 128, 32), np.float32)
    for c in range(2):
        for byte in range(128):
            full = 128 * c + byte
            h, l = full >> 4, full & 15
            w[c, byte, l] = 1.0  # cols 0..15 -> low sums -> stage 2p
            w[c, byte, 16 + h] = 1.0  # cols 16..31 -> high sums -> stage 2p+1
    return w


def _build():
    import concourse.bass as bass
    import concourse.bacc as bacc
    import concourse.mybir as mybir
    import concourse.tile as tile

    f32 = mybir.dt.float32
    f16 = mybir.dt.float16
    AF = mybir.ActivationFunctionType
    AX = mybir.AxisListType
    OP = mybir.AluOpType

    nc = bacc.Bacc("TRN2", target_bir_lowering=False, debug=False)
    at_d = nc.dram_tensor("at", [4, 2, P, B_CORE], f32, kind="ExternalInput")
    bt_d = nc.dram_tensor("bt", [4, 2, P, B_CORE], f32, kind="ExternalInput")
    w_d = nc.dram_tensor("w", [2, P, 32], f32, kind="ExternalInput")
    out_d = nc.dram_tensor("out", [B_CORE, 4, 256], f16, kind="ExternalOutput")

    def ap(base_ap, off, dims):
        part = base_ap.ap[0]
        return bass.AP(base_ap.tensor, base_ap.offset + off,
                       [list(part)] + [list(d) for d in dims])

    with tile.TileContext(nc) as tc:
        with (
            tc.tile_pool(name="persist", bufs=1) as pp,
            tc.tile_pool(name="pin", bufs=2) as pin,
            tc.tile_pool(name="pa", bufs=3) as pa,
            tc.tile_pool(name="pch", bufs=4) as pch,
            tc.tile_pool(name="pc", bufs=3) as pc,
            tc.tile_pool(name="pout", bufs=3) as pout,
            tc.psum_pool(name="ppsum", bufs=4) as pps,
        ):
            # ---------------- persistent tensors ----------------
            wsb = pp.tile([P, 2, 32], f32, tag="wsb")
            w_base = w_d.ap()
            nc.scalar.dma_start(
                wsb[:].rearrange("p c g -> p (c g)"),
                bass.AP(w_base.tensor, 0, [[32, P], [4096, 2], [1, 32]]))
            un_all = pp.tile([P, NT, 8, 18], f16, tag="un_all")
            msk = pp.tile([P, 256], f16, tag="msk")
            nc.gpsimd.memset(msk[:], 1.0)
            nc.gpsimd.memset(ap(msk[:], 0, [[16, 16]]), 0.0)
            nb100 = pp.tile([P, 1], f32, tag="nb100")
            nc.gpsimd.memset(nb100[:], -100.0)

            at_base = at_d.ap()
            bt_base = bt_d.ap()

            def in_view(base, row0, nrows):
                return bass.AP(
                    base.tensor, row0,
                    [[B_CORE, P], [2 * P * B_CORE, 4], [P * B_CORE, 2],
                     [1, nrows]])

            out_base = out_d.ap()

            # --------------- input loading ---------------
            def load_group(g, split_first=False):
                sl = pin.tile([P, 2, 4, 2, GT * P], f32, tag="slab")
                if split_first:
                    # per-tile loads so ST0's matmuls start asap
                    for h in range(4):
                        for tr, base in ((0, at_base), (1, bt_base)):
                            nc.sync.dma_start(
                                ap(sl[:], 4096 * tr + 128 * h,
                                   [[1024, 4], [512, 2], [1, 128]]),
                                in_view(base, GT * P * g + 128 * h, 128))
                else:
                    nc.sync.dma_start(
                        ap(sl[:], 0, [[1, 4096]]),
                        in_view(at_base, GT * P * g, GT * P))
                    nc.sync.dma_start(
                        ap(sl[:], 4096, [[1, 4096]]),
                        in_view(bt_base, GT * P * g, GT * P))
                return sl

            # --------------- phase A per super-tile (2 tiles) ---------------
            # a0: PE matmuls (one step early); a1r: c_sb/m16/ts; a1b: T exp;
            # a2: conv chain (one step late)
            def a0(u, sl):
                jg0 = (2 * u) % GT  # first tile index within load group
                c_ps = pps.tile([P, 2, 2, 8, 16], f32, tag="c")
                for j in range(2):
                    for tr in range(2):
                        for p4 in range(4):
                            for ch in range(2):
                                nc.tensor.matmul(
                                    ap(c_ps[:], 256 * tr + 128 * j + 32 * p4,
                                       [[16, 2], [1, 16]]),
                                    ap(sl[:], 4096 * tr + 1024 * p4 + 512 * ch
                                       + P * (jg0 + j), [[1, P]]),
                                    ap(wsb[:], 32 * ch, [[1, 32]]),
                                    start=(ch == 0), stop=(ch == 1))
                return c_ps

            def a1r_copy(u, c_ps):
                c_sb = pa.tile([P, 512], f32, tag="c_sb")
                nc.scalar.copy(
                    c_sb[:], c_ps[:].rearrange("p a b c e -> p (a b c e)"))
                return c_sb

            def a1r_rest(u, c_sb):
                m16 = pa.tile([P, 32], f32, tag="m16")
                nc.vector.tensor_reduce(
                    m16[:], c_sb[:].rearrange("p (g e) -> p g e", e=16),
                    axis=AX.X, op=OP.max)
                ts = pa.tile([P, 512], f32, tag="ts")
                nc.gpsimd.tensor_sub(
                    ts[:].rearrange("p (g e) -> p g e", e=16),
                    c_sb[:].rearrange("p (g e) -> p g e", e=16),
                    ap(m16[:], 0, [[1, 32], [0, 16]]))
                return ts

            def a1b(u, ts):
                T = pa.tile([P, 2, 2, 8, 16], f16, tag="T")
                nc.scalar.activation(
                    T[:].rearrange("p a b c e -> p (a b c e)"), ts[:],
                    AF.Exp, scale=100.0)
                return T

            def a2(u, T):
                # --- doubled xb + prefix sums (DVE) ---
                xbd = pa.tile([P, 2, 8, 32], f16, tag="xbd")
                nc.vector.tensor_copy(
                    xbd[:].rearrange("p a b (d e) -> p (a b) d e", d=2),
                    ap(T[:], 256, [[16, 16], [0, 2], [1, 16]]))
                p16 = pa.tile([P, 2, 8, 16], f16, tag="p16")
                nc.vector.tensor_tensor_scan(
                    p16[:].rearrange("p a b e -> p (a b e)"),
                    msk[:],
                    ap(T[:], 256, [[1, 256]]),
                    0.0, OP.mult, OP.add)
                # --- conv products q[(j,st) pair, m(17), i(16)] ---
                q = pa.tile([P, 16, 17, 16], f16, tag="q")
                qt1 = pa.tile([P, 16, 17, 8], f16, tag="qt1")
                qt2 = pa.tile([P, 16, 17, 4], f16, tag="qt2")
                qt3 = pa.tile([P, 16, 17, 2], f16, tag="qt3")
                qr = pa.tile([P, 16, 17], f16, tag="qr")
                nc.vector.tensor_mul(
                    ap(q[:], 0, [[272, 16], [16, 16], [1, 16]]),
                    ap(T[:], 0, [[16, 16], [0, 16], [1, 16]]),
                    ap(xbd[:], 16, [[32, 16], [1, 16], [-1, 16]]))
                nc.vector.tensor_mul(
                    ap(q[:], 256, [[272, 16], [1, 16]]),
                    ap(T[:], 0, [[16, 16], [1, 16]]),
                    ap(p16[:], 15, [[16, 16], [-1, 16]]))
                # --- tree sum over i ---
                nc.vector.tensor_add(
                    qt1[:],
                    ap(q[:], 0, [[272, 16], [16, 17], [1, 8]]),
                    ap(q[:], 8, [[272, 16], [16, 17], [1, 8]]))
                nc.vector.tensor_add(
                    qt2[:],
                    ap(qt1[:], 0, [[136, 16], [8, 17], [1, 4]]),
                    ap(qt1[:], 4, [[136, 16], [8, 17], [1, 4]]))
                nc.vector.tensor_add(
                    qt3[:],
                    ap(qt2[:], 0, [[68, 16], [4, 17], [1, 2]]),
                    ap(qt2[:], 2, [[68, 16], [4, 17], [1, 2]]))
                nc.vector.tensor_add(
                    qr[:],
                    ap(qt3[:], 0, [[34, 16], [2, 17]]),
                    ap(qt3[:], 1, [[34, 16], [2, 17]]))
                # --- normalize ---
                Su = pa.tile([P, 16], f32, tag="Su")
                nc.vector.tensor_reduce(
                    Su[:], ap(qr[:], 0, [[136, 2], [17, 8], [1, 16]]),
                    axis=AX.X, op=OP.add)
                rk = pa.tile([P, 16], f32, tag="rk")
                nc.vector.reciprocal(rk[:], Su[:])
                o = 288 * u
                nc.gpsimd.tensor_mul(
                    ap(un_all[:], o + 1, [[144, 2], [18, 8], [1, 17]]),
                    qr[:],
                    ap(rk[:], 0, [[8, 2], [1, 8], [0, 17]]))
                nc.gpsimd.tensor_mul(
                    ap(un_all[:], o, [[144, 2], [18, 8]]),
                    ap(qr[:], 15, [[136, 2], [17, 8]]),
                    ap(rk[:], 0, [[8, 2], [1, 8]]))

            # --------- carry chain for QT tiles [t0, t0+QT) ---------
            def chain(t0):
                o = 144 * t0
                n = 8 * QT
                wap = ap(un_all[:], o + 17, [[144, QT], [18, 8]])
                zap = ap(un_all[:], o + 16, [[144, QT], [18, 8]])
                wz = pch.tile([P, n], f16, tag="wz")
                v0a = pch.tile([P, n], f16, tag="v0a")
                bco = pch.tile([P, n], f16, tag="bco")
                gg = pch.tile([P, n + 1], f16, tag="gg")
                nc.gpsimd.tensor_sub(
                    wz[:].rearrange("p (a b) -> p a b", a=QT), wap, zap)
                nc.gpsimd.tensor_scalar(
                    v0a[:].rearrange("p (a b) -> p a b", a=QT), wap,
                    0.5, None, op0=OP.is_le)
                nc.gpsimd.tensor_scalar(
                    bco[:], wz[:], 0.5, None, op0=OP.is_le)
                nc.gpsimd.tensor_sub(
                    ap(bco[:], 1, [[8, QT], [1, 7]]),
                    ap(bco[:], 1, [[8, QT], [1, 7]]),
                    ap(v0a[:], 1, [[8, QT], [1, 7]]))
                nc.gpsimd.memset(ap(bco[:], 0, [[8, QT]]), 0.0)
                nc.vector.tensor_tensor_scan(
                    ap(gg[:], 1, [[1, n]]),
                    bco[:], v0a[:], 0.0, OP.mult, OP.add)
                nc.gpsimd.memset(ap(gg[:], 0, [[8, QT]]), 0.0)
                g16 = pch.tile([P, QT, 8, 16], f16, tag="g16")
                nc.scalar.copy(
                    g16[:].rearrange("p a b e -> p (a b e)"),
                    ap(gg[:], 0, [[8, QT], [1, 8], [0, 16]]))
                return g16

            # --------- phase C per super-tile, 4 pipelined segments ---------
            obufs = {}

            def c_a(u, g16):
                o = 288 * u
                dlt = pc.tile([P, 2, 8, 16], f16, tag="dlt")
                nc.gpsimd.tensor_sub(
                    dlt[:],
                    ap(un_all[:], o, [[144, 2], [18, 8], [1, 16]]),
                    ap(un_all[:], o + 1, [[144, 2], [18, 8], [1, 16]]))
                tb2 = pc.tile([P, 2, 8, 16], f16, tag="tb2")
                tau = (2 * u) % QT
                nc.gpsimd.tensor_mul(
                    tb2[:], dlt[:],
                    ap(g16[:], 128 * tau, [[128, 2], [16, 8], [1, 16]]))
                sb = pc.tile([P, 2, 8, 16], f16, tag="sb")
                nc.gpsimd.tensor_add(
                    sb[:], tb2[:],
                    ap(un_all[:], o + 1, [[144, 2], [18, 8], [1, 16]]))
                eh = pc.tile([P, 2, 8, 16], f32, tag="eh")
                nc.scalar.activation(
                    eh[:].rearrange("p a b e -> p (a b e)"),
                    sb[:].rearrange("p a b e -> p (a b e)"),
                    AF.Exp, bias=nb100[:], scale=100.0)
                return eh

            def c_b(u, eh):
                ns = pc.tile([P, 16], f32, tag="ns")
                nc.vector.tensor_reduce(
                    ns[:], eh[:].rearrange("p a b e -> p (a b) e"),
                    axis=AX.X, op=OP.add)
                r1 = pc.tile([P, 16], f32, tag="r1")
                nc.vector.reciprocal(r1[:], ns[:])
                A16 = pc.tile([P, 2, 8, 16], f16, tag="A16")
                nc.gpsimd.tensor_mul(
                    A16[:], eh[:], ap(r1[:], 0, [[8, 2], [1, 8], [0, 16]]))
                e2 = pc.tile([P, 2, 8, 16], f32, tag="e2")
                nc.scalar.activation(
                    e2[:].rearrange("p a b e -> p (a b e)"),
                    A16[:].rearrange("p a b e -> p (a b e)"),
                    AF.Exp, bias=nb100[:], scale=100.0)
                return e2

            def c_c(u, e2):
                s2 = pc.tile([P, 16], f32, tag="s2")
                nc.vector.tensor_reduce(
                    s2[:], e2[:].rearrange("p a b e -> p (a b) e"),
                    axis=AX.X, op=OP.add)
                r2 = pc.tile([P, 16], f32, tag="r2")
                nc.vector.reciprocal(r2[:], s2[:])
                e2l = pc.tile([P, 2, 4, 16], f16, tag="e2l")
                nc.gpsimd.tensor_mul(
                    e2l[:],
                    ap(e2[:], 0, [[128, 2], [32, 4], [1, 16]]),
                    ap(r2[:], 0, [[8, 2], [2, 4], [0, 16]]))
                e2h = pc.tile([P, 2, 4, 16], f16, tag="e2h")
                nc.gpsimd.tensor_mul(
                    e2h[:],
                    ap(e2[:], 16, [[128, 2], [32, 4], [1, 16]]),
                    ap(r2[:], 1, [[8, 2], [2, 4], [0, 16]]))
                rep = pc.tile([P, 2, 4, 16, 16], f16, tag="rep")
                nc.scalar.copy(
                    rep[:].rearrange("p a b h l -> p (a b h l)"),
                    ap(e2h[:], 0, [[64, 2], [16, 4], [1, 16], [0, 16]]))
                return rep, e2l

            def c_d(u, rep, e2l):
                if u % 2 == 0:
                    o_t = pout.tile([P, GT, 4, 16, 16], f16, tag="o_t")
                    obufs[u // 2] = o_t
                o_t = obufs[u // 2]
                nc.vector.tensor_mul(
                    ap(o_t[:], 2048 * (u % 2),
                       [[1024, 2], [256, 4], [16, 16], [1, 16]]),
                    rep[:],
                    ap(e2l[:], 0, [[64, 2], [16, 4], [0, 16], [1, 16]]))
                if u % 2 == 1:
                    t0 = 2 * u - 2
                    if u == NST - 1:
                        # split the last store so the tail drains sooner
                        for h in range(2):
                            dv = bass.AP(
                                out_base.tensor, 1024 * P * (t0 + 2 * h),
                                [[1024, P], [1024 * P, 2], [1, 1024]])
                            nc.scalar.dma_start(
                                dv, ap(o_t[:], 2048 * h, [[1, 2048]]))
                    else:
                        dview = bass.AP(
                            out_base.tensor, 1024 * P * t0,
                            [[1024, P], [1024 * P, GT], [1, 1024]])
                        nc.scalar.dma_start(
                            dview,
                            o_t[:].rearrange("p a b h l -> p (a b h l)"))

            # ================= emission schedule =================
            # lags: a1(u) this step; T(u-1) + a2(u-1) next step; chain after
            # a2 of each odd ST; phase_c three steps behind a1.
            g16s = {}
            slabs = {0: load_group(0, split_first=True)}
            c_pss = {0: a0(0, slabs[0])}
            pend = None  # (u, ts) awaiting a1b/a2
            ehs, e2s, reps = {}, {}, {}
            for u in range(NST + 6):
                if pend is not None and pend[0] < NST:
                    pT = a1b(pend[0], pend[1])  # ACT: T(u-1), ready
                if u < NST:
                    csb_u = a1r_copy(u, c_pss.pop(u))  # ACT: c_copy early
                if u - 6 >= 0:
                    c_d(u - 6, *reps.pop(u - 6))
                if u - 5 >= 0 and u - 5 < NST:
                    reps[u - 5] = c_c(u - 5, e2s.pop(u - 5))
                if u - 4 >= 0 and u - 4 < NST:
                    e2s[u - 4] = c_b(u - 4, ehs.pop(u - 4))
                if u < NST:
                    ts_u = a1r_rest(u, csb_u)  # DVE m16; Pool ts
                if u - 3 >= 0 and u - 3 < NST:
                    ehs[u - 3] = c_a(u - 3, g16s[(u - 3) // 2])
                if pend is not None and pend[0] < NST:
                    a2(pend[0], pT)
                    if pend[0] % 2 == 1:
                        t0 = 2 * pend[0] - 2
                        g16s[t0 // QT] = chain(t0)
                    pend = None
                if u < NST:
                    g = (2 * u) // GT
                    if (2 * u) % GT == 0 and g + 1 < NG:
                        slabs[g + 1] = load_group(g + 1)
                    if u + 1 < NST:
                        gn = (2 * (u + 1)) // GT
                        c_pss[u + 1] = a0(u + 1, slabs[gn])
                    pend = (u, ts_u)

    nc.compile()
    return nc


def _get_nc():
    global _BUILT
    if _BUILT is None:
        _BUILT = _build()
    return _BUILT


def kernel(a, b, add_table=None, carry_table=None, b2n=None, n2b=None, **_kw):
    from concourse.bass_utils import run_bass_kernel_spmd

    a = np.asarray(a, dtype=np.float32).reshape(B_FULL, 4, 256)
    b = np.asarray(b, dtype=np.float32).reshape(B_FULL, 4, 256)
    w = _make_w()
    nc = _get_nc()
    in_maps = []
    for i in range(N_CORES):
        sl = slice(i * B_CORE, (i + 1) * B_CORE)
        # [rows, pos, 256] -> [pos, chunk, byte(128), rows]
        at = np.ascontiguousarray(
            a[sl].reshape(B_CORE, 4, 2, P).transpose(1, 2, 3, 0))
        bt = np.ascontiguousarray(
            b[sl].reshape(B_CORE, 4, 2, P).transpose(1, 2, 3, 0))
        in_maps.append({"at": at, "bt": bt, "w": w})
    res = run_bass_kernel_spmd(nc, in_maps, core_ids=list(range(N_CORES)))
    out = np.concatenate([r["out"] for r in res.results], axis=0)
    return out.astype(np.float32)
